# revision 15
# baseline (speedup 1.0000x reference)
"""DLRM DotInteraction kernel for Trainium2 (Bass/Tile), 8-core data parallel.

Problem: dense_feature [B=16384, D=128] f32, sparse_stack [S=26, B, D] f32.
cat = [dense; sparse] per sample -> [B, N=27, D]; G_b = cat_b @ cat_b^T;
out = [dense | tril(G_b) (378 vals, row-major incl diag)] -> [B, 506] f32.

Per core (B_c = 2048 samples), pipelined over supertiles of 4x128-sample tiles:
  1. Input f32 -> f16 via SWDGE cast-DMA (all tiles; gpsimd queue).
  2. Transpose [128 s, 128 d] -> [128 d, 128 s] per feature slab on TensorE
     as a REGULAR matmul (lhsT=slab, rhs=identity): out = slab.T @ I. Unlike
     transpose-mode, the 128-col fp16 weight load is FWL-eligible. PSUM out
     is f32; the DVE/ScalarE copy-back casts to f16 xt.
  3. TensorE Gram: group g = samples {32c + g}; one explicit full-array
     ldweights [128 d, 128] covering all 4 samples (FWL), then 4 col-tiled
     matmuls (tile_position (0,32c), M=32, N=27) whose auto-paired redundant
     32-col ldweights are deleted post-compile (see _strip_gram_ldweights).
  4. DVE/ScalarE copy Gram PSUM (f32) -> SBUF gcol f16 [32c+i, g, tp, j].
  5. Flatten: 27 DMAs per supertile gather Gram rows into output-row tiles
     rowq f16 [sample partition, 506]; dense cols copied from nat (f16);
     one SWDGE cast store (f16 -> f32) per 128-sample tile.
"""

import numpy as np

import concourse.bacc as bacc
import concourse.mybir as mybir
import concourse.tile as tile
from concourse import bass_utils
from concourse.masks import make_identity

B = 16384
D = 128
S = 26
N = S + 1  # 27
NCORES = 8
BC = B // NCORES  # 2048 samples per core
PT = 128  # samples per sbuf tile
GPR = 16  # groups per psum round
TRI = N * (N + 1) // 2  # 378
W = D + TRI  # 506
TPS = 4  # tiles per supertile

f32 = mybir.dt.float32
f16 = mybir.dt.float16

# variant flags (A/B testing)
FUSE_GRAM_LDW = False  # (failed variant: full-mode LDW + col-tiled MMs)
MM_TRANSPOSE = True  # transpose via regular matmul vs transpose-mode
GRAM_LDW_ORDER = "cf"  # full-ldweights column iteration: "cf" or "fc"
GRAM_LDW_POS = None  # tile_position for the full ldweights
GRAM_FUSED_FULL = True  # one full-mode self-loading matmul per 4-sample group


def _ap_free_size(ap) -> int:
    n = 1
    for stride_count in ap.ap[1:]:
        n *= stride_count[1]
    return n


def _strip_gram_ldweights(nc):
    """Delete the auto-paired 32-col InstLdweights of col-tiled Gram matmuls.

    The legalizer splits every InstMatmult into InstLdweights + InstMatmult
    (ldweights=False) unconditionally. The Gram groups already start with an
    explicit full-array [128, 4*32] ldweights covering all four quadrants, so
    the per-matmul 32-col loads are redundant. They carry no sync (all sem
    protocol lives in separate InstEventSemaphore instructions), so deleting
    them is safe. Discriminator: weights-AP free size == 32 (all other
    ldweights in this kernel load 128 columns).
    """
    removed = 0
    for b in nc.m.functions[0].blocks:
        insts = b.instructions
        keep = []
        for i in insts:
            if type(i).__name__ == "InstLdweights" and _ap_free_size(i.ins[0]) == 32:
                s = getattr(i, "sync", None)
                assert not (s and (s.waits or s.updates)), "ldweights has sync"
                removed += 1
                continue
            keep.append(i)
        if removed:
            insts[:] = keep
    return removed


def build_kernel(b_core: int = BC, reps: int = 1):
    nc = bacc.Bacc("TRN2", target_bir_lowering=False, debug=False)
    dense = nc.dram_tensor("dense", [b_core, D], f32, kind="ExternalInput").ap()
    sparse = nc.dram_tensor("sparse", [S, b_core, D], f32, kind="ExternalInput").ap()
    out = nc.dram_tensor("out", [b_core, W], f32, kind="ExternalOutput").ap()

    t_total = b_core // PT
    gpt = PT // 4  # 32 groups per tile
    gpr = 8 if GRAM_FUSED_FULL else GPR  # groups per psum round
    rpt = gpt // gpr  # psum rounds per tile
    tps = min(TPS, t_total)
    n_super = t_total // tps
    psumt_bufs = 2 if GRAM_FUSED_FULL else 3

    with tile.TileContext(nc) as tc:
        with (
            tc.tile_pool(name="singles", bufs=1) as singles,
            tc.tile_pool(name="nat", bufs=2) as nat_pool,
            tc.tile_pool(name="xt", bufs=3) as xt_pool,
            tc.tile_pool(name="gcol", bufs=2) as gcol_pool,
            tc.tile_pool(name="row", bufs=2) as row_pool,
            tc.tile_pool(name="psum", bufs=3, space="PSUM") as psum_pool,
            tc.tile_pool(name="psumt", bufs=psumt_bufs, space="PSUM") as psumt_pool,
        ):
            id16 = singles.tile([128, 128], f16, name="id16")
            make_identity(nc, id16)

            for _rep in range(reps):
                for st in range(n_super):
                    # gcol[32c+i, g, tp, j] = Gram[i,j] of sample 32c+g in
                    # tile tp of this supertile (f16).
                    gcol = gcol_pool.tile([32, 4, gpt, tps, N], f16)
                    # rowq[p, tp, :] = output row of sample (st, tp, p) (f16)
                    rowq = row_pool.tile([128, tps, W], f16)

                    # --- supertile load + cast to fp16: nat[s, tp, j, d] ---
                    # (dense batched per supertile; sparse per tile — DMA APs
                    # are limited to 3 dims)
                    srows = slice(st * tps * PT, (st + 1) * tps * PT)
                    nat = nat_pool.tile([128, tps, N, D], f16)
                    nc.gpsimd.dma_start(
                        out=nat[:, :, 0, :],
                        in_=dense[srows, :].rearrange("(tp b) d -> b tp d", b=PT),
                    )
                    for tp in range(tps):
                        t = st * tps + tp
                        rows = slice(t * PT, (t + 1) * PT)
                        nc.gpsimd.dma_start(
                            out=nat[:, tp, 1:N, :],
                            in_=sparse[:, rows, :].rearrange("s b d -> b s d"),
                        )

                    for tp in range(tps):
                        # dense passthrough columns (f16 copy from nat)
                        nc.scalar.copy(out=rowq[:, tp, 0:D], in_=nat[:, tp, 0, :])

                        # --- TensorE transpose of each feature slab ---
                        # xt[d, g, c, f] = cat[sample 32c+g, feature f, d]:
                        # each group's 4*32 weight columns are contiguous so
                        # the fused Gram matmul's weights AP has one free dim
                        # (walrus requirement). f 27:32 is zero padding.
                        xt = xt_pool.tile([128, gpt, 4, 32], f16)
                        nc.gpsimd.memset(xt[:, :, :, N:32], 0.0)
                        for k in range(7):  # 4-slab packs: 6*4 + 3
                            j0 = 4 * k
                            nj = min(4, N - j0)
                            if MM_TRANSPOSE:
                                ptf = psumt_pool.tile([128, 4, PT], f32, tag="pt")
                                for jj in range(nj):
                                    nc.tensor.matmul(
                                        out=ptf[:, jj, :],
                                        lhsT=nat[:, tp, j0 + jj, :],
                                        rhs=id16[:, :],
                                        start=True,
                                        stop=True,
                                    )
                            else:
                                ptf = psumt_pool.tile([128, 4, PT], f16, tag="pt")
                                for jj in range(nj):
                                    nc.tensor.transpose(
                                        ptf[:, jj, :], nat[:, tp, j0 + jj, :], id16
                                    )
                            cp = nc.vector.tensor_copy if k % 2 else nc.scalar.copy
                            cp(
                                out=xt[:, :, :, j0 : j0 + nj].rearrange(
                                    "p g c f -> p f c g"
                                ),
                                in_=ptf[:, 0:nj, :].rearrange(
                                    "p f (c g) -> p f c g", c=4
                                ),
                            )

                        # --- Gram matmuls ---
                        if GRAM_FUSED_FULL:
                            # one full-mode matmul per 4-sample group: lhsT
                            # [128 d, (c,f)=128] (FWL), rhs [128 d, (c,j)=108].
                            # out[32c'+i, 27c+j] valid iff c'==c; the copies
                            # extract the diagonal blocks. PSUM slot padded to
                            # 128 f32 so 4 slots fill a bank exactly.
                            for r in range(rpt):
                                ps = psum_pool.tile([128, gpr, 128], f32)
                                for q in range(gpr):
                                    g_local = r * gpr + q
                                    nc.tensor.matmul(
                                        out=ps[:, q, 0 : 4 * N],
                                        lhsT=xt[:, g_local, :, :],
                                        rhs=xt[:, g_local, :, 0:N],
                                        start=True,
                                        stop=True,
                                    )
                                off = r * gpr
                                for c in range(4):
                                    cp2 = (
                                        nc.vector.tensor_copy
                                        if c % 2
                                        else nc.scalar.copy
                                    )
                                    cp2(
                                        out=gcol[:, c, off : off + gpr, tp, :],
                                        in_=ps[
                                            32 * c : 32 * c + 32,
                                            :,
                                            N * c : N * c + N,
                                        ],
                                    )
                        else:
                            for r in range(rpt):
                                ps = psum_pool.tile([128, gpr, N], f32)
                                for q in range(gpr):
                                    g_local = r * gpr + q
                                    for c in range(4):
                                        nc.tensor.matmul(
                                            out=ps[32 * c : 32 * c + 32, q, :],
                                            lhsT=xt[:, g_local, c, :],
                                            rhs=xt[:, g_local, c, 0:N],
                                            start=True,
                                            stop=True,
                                            tile_position=(0, 32 * c),
                                        )
                                off = r * gpr
                                for c in range(4):
                                    cp2 = (
                                        nc.vector.tensor_copy
                                        if c % 2
                                        else nc.scalar.copy
                                    )
                                    cp2(
                                        out=gcol[:, c, off : off + gpr, tp, :],
                                        in_=ps[32 * c : 32 * c + 32, :, :],
                                    )

                    # --- flatten: Gram row i of sample (tp, 32c+g) from
                    # gcol[32c+i, g, tp, 0:i+1] to rowq[32c+g, tp, toff:] ---
                    for i in range(N):
                        toff = D + i * (i + 1) // 2
                        eng = nc.scalar if i % 3 == 0 else nc.sync
                        eng.dma_start(
                            # write iter ((c,g)->partition, tp, j)
                            out=rowq[:, :, toff : toff + i + 1],
                            # read iter (c, g, tp, j) on partition i
                            in_=gcol[i : i + 1, :, :, :, 0 : i + 1],
                        )

                    # --- store: SWDGE cast f16 -> f32, whole supertile ---
                    nc.gpsimd.dma_start(
                        out=out[srows, :].rearrange("(tp b) w -> b tp w", b=PT),
                        in_=rowq[:, :, :],
                    )

    nc.compile()
    if FUSE_GRAM_LDW:
        _strip_gram_ldweights(nc)
    return nc


_CACHE: dict = {}


def _get_nc():
    if "nc" not in _CACHE:
        _CACHE["nc"] = build_kernel(BC)
    return _CACHE["nc"]


def kernel(dense_feature, sparse_stack, **run_kwargs):
    dense_feature = np.asarray(dense_feature, dtype=np.float32)
    sparse_stack = np.asarray(sparse_stack, dtype=np.float32)
    assert dense_feature.shape == (B, D)
    assert sparse_stack.shape == (S, B, D)

    nc = run_kwargs.pop("nc", None) or _get_nc()
    in_maps = []
    for ci in range(NCORES):
        sl = slice(ci * BC, (ci + 1) * BC)
        in_maps.append(
            {
                "dense": np.ascontiguousarray(dense_feature[sl]),
                "sparse": np.ascontiguousarray(sparse_stack[:, sl, :]),
            }
        )
    res = bass_utils.run_bass_kernel_spmd(
        nc, in_maps, core_ids=list(range(NCORES)), **run_kwargs
    )
    out = np.concatenate([r["out"] for r in res.results], axis=0)
    if run_kwargs:
        _CACHE["last_result"] = res
    return out


# revision 17
# speedup vs baseline: 1.4034x; 1.4034x over previous
"""DLRM DotInteraction kernel for Trainium2 (Bass/Tile), 8-core data parallel.

Problem: dense_feature [B=16384, D=128] f32, sparse_stack [S=26, B, D] f32.
cat = [dense; sparse] per sample -> [B, N=27, D]; G_b = cat_b @ cat_b^T;
out = [dense | tril(G_b) (378 vals, row-major incl diag)] -> [B, 506] f32.

Per core (B_c = 2048 samples), pipelined over supertiles of 4x128-sample tiles:
  1. Input f32 -> f16 via SWDGE cast-DMA (gpsimd queue): dense batched per
     supertile, sparse per tile (DMA APs are limited to 3 dims).
  2. Transpose [128 s, 128 d] -> [128 d, 128 s] per feature slab on TensorE
     as a REGULAR matmul (lhsT=slab, rhs=identity): out = slab.T @ I. PSUM
     out is f32; the DVE/ScalarE copy-back casts to f16 xt [d, feat, s].
  3. TensorE Gram: group g = samples {32c + g}; 4 col-tiled matmuls per group
     (tile_position (0,32c)), K=128 (d), M=27 (no pad), N=27, f32 PSUM.
  4. DVE/ScalarE copy Gram PSUM (f32) -> SBUF gcol f16 [i, c, g, tp, j].
  5. Flatten: 27 DMAs per supertile gather Gram rows into output-row tiles
     rowq f16 [sample partition, 506]; dense cols copied from nat (f16);
     one SWDGE cast store (f16 -> f32) per supertile.
"""

import numpy as np

import concourse.bacc as bacc
import concourse.mybir as mybir
import concourse.tile as tile
from concourse import bass_utils
from concourse.masks import make_identity

B = 16384
D = 128
S = 26
N = S + 1  # 27
NCORES = 8
BC = B // NCORES  # 2048 samples per core
PT = 128  # samples per sbuf tile
GPR = 16  # groups per psum round
TRI = N * (N + 1) // 2  # 378
W = D + TRI  # 506
TPS = 4  # tiles per supertile

f32 = mybir.dt.float32
f16 = mybir.dt.float16

MM_TRANSPOSE = True  # transpose via regular matmul vs transpose-mode


def build_kernel(b_core: int = BC, reps: int = 1):
    nc = bacc.Bacc("TRN2", target_bir_lowering=False, debug=False)
    dense = nc.dram_tensor("dense", [b_core, D], f32, kind="ExternalInput").ap()
    sparse = nc.dram_tensor("sparse", [S, b_core, D], f32, kind="ExternalInput").ap()
    out = nc.dram_tensor("out", [b_core, W], f32, kind="ExternalOutput").ap()

    t_total = b_core // PT
    gpt = PT // 4  # 32 groups per tile
    rpt = gpt // GPR  # psum rounds per tile
    tps = min(TPS, t_total)
    n_super = t_total // tps

    with tile.TileContext(nc) as tc:
        with (
            tc.tile_pool(name="singles", bufs=1) as singles,
            tc.tile_pool(name="nat", bufs=2) as nat_pool,
            tc.tile_pool(name="xt", bufs=3) as xt_pool,
            tc.tile_pool(name="gcol", bufs=2) as gcol_pool,
            tc.tile_pool(name="row", bufs=2) as row_pool,
            tc.tile_pool(name="psum", bufs=4, space="PSUM") as psum_pool,
            tc.tile_pool(name="psumt", bufs=4, space="PSUM") as psumt_pool,
        ):
            id16 = singles.tile([128, 128], f16, name="id16")
            make_identity(nc, id16)

            for _rep in range(reps):
                for st in range(n_super):
                    # gcol[i, c, g, tp, j] = Gram[i,j] of sample 32c+g in
                    # tile tp of this supertile (f16).
                    gcol = gcol_pool.tile([32, 4, gpt, tps, N], f16)
                    # rowq[p, tp, :] = output row of sample (st, tp, p) (f16)
                    rowq = row_pool.tile([128, tps, W], f16)

                    # --- supertile load + cast to fp16: nat[s, tp, j, d] ---
                    srows = slice(st * tps * PT, (st + 1) * tps * PT)
                    nat = nat_pool.tile([128, tps, N, D], f16)
                    nc.gpsimd.dma_start(
                        out=nat[:, :, 0, :],
                        in_=dense[srows, :].rearrange("(tp b) d -> b tp d", b=PT),
                    )
                    for tp in range(tps):
                        t = st * tps + tp
                        rows = slice(t * PT, (t + 1) * PT)
                        nc.gpsimd.dma_start(
                            out=nat[:, tp, 1:N, :],
                            in_=sparse[:, rows, :].rearrange("s b d -> b s d"),
                        )

                    for tp in range(tps):
                        # dense passthrough columns (f16 copy from nat)
                        nc.scalar.copy(out=rowq[:, tp, 0:D], in_=nat[:, tp, 0, :])

                        # --- TensorE transpose of each feature slab ---
                        xt = xt_pool.tile([128, N, PT], f16)
                        for k in range(7):  # 4-slab packs: 6*4 + 3
                            j0 = 4 * k
                            nj = min(4, N - j0)
                            if MM_TRANSPOSE:
                                ptf = psumt_pool.tile([128, 4, PT], f32, tag="pt")
                                for jj in range(nj):
                                    nc.tensor.matmul(
                                        out=ptf[:, jj, :],
                                        lhsT=nat[:, tp, j0 + jj, :],
                                        rhs=id16[:, :],
                                        start=True,
                                        stop=True,
                                    )
                            else:
                                ptf = psumt_pool.tile([128, 4, PT], f16, tag="pt")
                                for jj in range(nj):
                                    nc.tensor.transpose(
                                        ptf[:, jj, :], nat[:, tp, j0 + jj, :], id16
                                    )
                            cp = nc.vector.tensor_copy if k % 2 else nc.scalar.copy
                            cp(out=xt[:, j0 : j0 + nj, :], in_=ptf[:, 0:nj, :])

                        # --- Gram matmuls: M=27, no pad ---
                        for r in range(rpt):
                            ps = psum_pool.tile([128, GPR, N], f32)
                            for q in range(GPR):
                                g_local = r * GPR + q
                                for c in range(4):
                                    s_local = 32 * c + g_local
                                    nc.tensor.matmul(
                                        out=ps[32 * c : 32 * c + N, q, :],
                                        lhsT=xt[:, :, s_local],
                                        rhs=xt[:, :, s_local],
                                        start=True,
                                        stop=True,
                                        tile_position=(0, 32 * c),
                                    )
                            off = r * GPR
                            for c in range(4):
                                cp2 = nc.vector.tensor_copy if c % 2 else nc.scalar.copy
                                cp2(
                                    out=gcol[0:N, c, off : off + GPR, tp, :],
                                    in_=ps[32 * c : 32 * c + N, :, :],
                                )

                    # --- flatten: Gram row i of sample (tp, 32c+g) from
                    # gcol[i, c, g, tp, 0:i+1] to rowq[32c+g, tp, toff:] ---
                    for i in range(N):
                        toff = D + i * (i + 1) // 2
                        eng = nc.scalar if i % 3 == 0 else nc.sync
                        eng.dma_start(
                            # write iter ((c,g)->partition, tp, j)
                            out=rowq[:, :, toff : toff + i + 1],
                            # read iter (c, g, tp, j) on partition i
                            in_=gcol[i : i + 1, :, :, :, 0 : i + 1],
                        )

                    # --- store: SWDGE cast f16 -> f32, whole supertile ---
                    nc.gpsimd.dma_start(
                        out=out[srows, :].rearrange("(tp b) w -> b tp w", b=PT),
                        in_=rowq[:, :, :],
                    )

    nc.compile()
    return nc


_CACHE: dict = {}


def _get_nc():
    if "nc" not in _CACHE:
        _CACHE["nc"] = build_kernel(BC)
    return _CACHE["nc"]


def kernel(dense_feature, sparse_stack, **run_kwargs):
    dense_feature = np.asarray(dense_feature, dtype=np.float32)
    sparse_stack = np.asarray(sparse_stack, dtype=np.float32)
    assert dense_feature.shape == (B, D)
    assert sparse_stack.shape == (S, B, D)

    nc = run_kwargs.pop("nc", None) or _get_nc()
    in_maps = []
    for ci in range(NCORES):
        sl = slice(ci * BC, (ci + 1) * BC)
        in_maps.append(
            {
                "dense": np.ascontiguousarray(dense_feature[sl]),
                "sparse": np.ascontiguousarray(sparse_stack[:, sl, :]),
            }
        )
    res = bass_utils.run_bass_kernel_spmd(
        nc, in_maps, core_ids=list(range(NCORES)), **run_kwargs
    )
    out = np.concatenate([r["out"] for r in res.results], axis=0)
    if run_kwargs:
        _CACHE["last_result"] = res
    return out
